# revision 11
# baseline (speedup 1.0000x reference)
"""8-bit ripple-carry adder on {0,1} floats — Trainium2 Bass kernel.

Problem: A, B [N=2^23, 8] f32 bits (MSB first), Cin [N,1] f32.
reference ripples from bit 7 (LSB) to bit 0 (MSB):
    t = a + b + c ; s = t mod 2 ; c' = t >= 2
Returns (sums [N,8], carry [N,1]) like the reference.

Sharding: batch dim N split evenly across 8 NeuronCores, no communication.

Host side packs A|B row-wise into one [NS,16] tensor per core so each chunk
is a single DMA (the TT ISA slot supports only one sync wait; two separate
loads would land on two DMAHW lanes and need two waits on the consumer).

Per-core layout: rows are processed in chunks of 128*R rows. A chunk of AB
loads contiguously into an SBUF tile [128, 16R] (partition p holds R full
rows). Bit i of each row: A_i = tile[:, i::16], B_i = tile[:, 8+i::16].

Per bit (DVE = vector, ACT = scalar engine):
    ts = A_i + B_i            (DVE tensor_tensor, strided reads)
    t2 = ts + carry           (DVE tensor_tensor, compact)
    carry = t2 >= 2           (DVE tensor_scalar)
    s_i = |sin(pi/2 * t2)|    (ACT sin then abs; exact-enough on {0,1,2,3})
"""

import math
import os

import numpy as np

N_TOTAL = 8388608
N_CORES = 8
NS = N_TOTAL // N_CORES  # rows per core

F32_R = 512  # rows per partition per chunk (f32 path)

_CACHE = {}


def _build_f32(R: int):
    """Wait-slot-safe pipeline (HW compute instructions fit ONE sync wait).

    Tricks:
      - A|B packed host-side into one [NS,16] tensor -> one load DMA/chunk.
      - bf16 intermediates unlock DVE 2x/4x perf modes (values in {0..3}
        are exact in bf16); I/O stays f32.
      - "primer" ops (tiny memset / 16-col ACT copy) absorb WAR-vs-store
        and WAR-vs-other-engine waits so every real op carries only its
        RAW wait. Engine-level sem observation then covers later ops.
    """
    import concourse.tile as tile
    from concourse import bacc, mybir

    f32 = mybir.dt.float32
    bf16 = mybir.dt.bfloat16
    chunk_rows = 128 * R
    n_chunks = NS // chunk_rows
    assert NS % chunk_rows == 0

    nc = bacc.Bacc(None)
    AB = nc.declare_dram_parameter("AB", [NS, 16], f32, isOutput=False)
    Cin = nc.declare_dram_parameter("Cin", [NS, 1], f32, isOutput=False)
    S = nc.declare_dram_parameter("sums", [NS, 8], f32, isOutput=True)
    CO = nc.declare_dram_parameter("carry", [NS, 1], f32, isOutput=True)

    ABv = AB[:].rearrange("(c p r) m -> c p (r m)", p=128, r=R)
    Cv = Cin[:].rearrange("(c p r) m -> c p (r m)", p=128, r=R)
    Sv = S[:].rearrange("(c p r) m -> c p (r m)", p=128, r=R)
    COv = CO[:].rearrange("(c p r) m -> c p (r m)", p=128, r=R)

    HALF_PI = math.pi / 2.0
    Sin = mybir.ActivationFunctionType.Sin
    Abs = mybir.ActivationFunctionType.Abs
    is_ge = mybir.AluOpType.is_ge

    with tile.TileContext(nc) as tc:
        with (
            tc.tile_pool(name="const", bufs=1) as const_pool,
            tc.tile_pool(name="io", bufs=3) as io_pool,
            tc.tile_pool(name="tmp", bufs=3) as tmp_pool,
            tc.tile_pool(name="tsp", bufs=9) as ts_pool,
        ):
            z16 = const_pool.tile([128, 16], f32, tag="z16")
            nc.vector.memset(z16[:], 0.0)
            # Sin bias: shift inputs {0..3} to [-pi, pi/2] where the ACT
            # spline is accurate (sin(3*pi/2) evaluates to 0.9248 raw).
            npi = const_pool.tile([128, 1], f32, tag="npi")
            nc.vector.memset(npi[:], -math.pi)

            for c in range(n_chunks):
                tAB = io_pool.tile([128, 16 * R], f32, tag="AB")
                nc.sync.dma_start(out=tAB[:], in_=ABv[c])
                tC = io_pool.tile([128, R], f32, tag="Cin")
                nc.sync.dma_start(out=tC[:], in_=Cv[c])
                tOUT = io_pool.tile([128, 8 * R], f32, tag="OUT")

                # ACT-side primer: absorbs tOUT's WAR-vs-sums-store wait and
                # WAW-overlaps every abs write region (cols 0..15 hit every
                # i::8 slice).
                nc.scalar.copy(tOUT[:][:, 0:16], z16[:])

                # t2 segments live in one tile; one strided memset absorbs
                # the WAR-vs-sin (ACT) wait for all 8 segments.
                t2a = tmp_pool.tile([128, 8 * R], bf16, tag="t2")
                nc.vector.memset(t2a[:][:, 0 :: R], 0.0)

                # final-carry slot: primer absorbs WAR-vs-carry-store.
                cnf = tmp_pool.tile([128, R], f32, tag="cnf")
                nc.vector.memset(cnf[:][:, 0:1], 0.0)

                # Real data dep on the Cin DMA (carries its DMA wait).
                carry = tmp_pool.tile([128, R], bf16, tag="c0")
                nc.vector.tensor_copy(carry[:], tC[:])

                # All 8 strided A_i+B_i adds first: releases the big AB tile
                # as early as possible so the next chunk's load can start.
                tss = []
                for i in [7, 6, 5, 4, 3, 2, 1, 0]:
                    ts = ts_pool.tile([128, R], bf16, tag="ts")
                    nc.vector.tensor_add(
                        ts[:], tAB[:][:, i::16], tAB[:][:, 8 + i :: 16]
                    )
                    tss.append(ts)

                for k, i in enumerate([7, 6, 5, 4, 3, 2, 1, 0]):
                    ts = tss[k]
                    t2 = t2a[:][:, k * R : (k + 1) * R]
                    nc.vector.tensor_add(t2, ts[:], carry[:])
                    if i > 0:
                        cn = tmp_pool.tile([128, R], bf16, tag="cn")
                    else:
                        cn = cnf
                    nc.vector.tensor_scalar(cn[:], t2, 2.0, None, is_ge)
                    sr = tmp_pool.tile([128, R], bf16, tag="sr")
                    nc.scalar.activation(
                        sr[:], t2, Sin, scale=HALF_PI, bias=npi[:]
                    )
                    nc.scalar.activation(tOUT[:][:, i::8], sr[:], Abs)
                    carry = cn

                nc.sync.dma_start(out=Sv[c], in_=tOUT[:])
                nc.sync.dma_start(out=COv[c], in_=cnf[:])
    nc.finalize()
    return nc


def _get_nc():
    key = ("f32", F32_R)
    if key not in _CACHE:
        _CACHE[key] = _build_f32(F32_R)
    return _CACHE[key]


def kernel(A, B, Cin, _trace=False):
    from concourse.bass_utils import run_bass_kernel_spmd

    A = np.asarray(A, dtype=np.float32)
    B = np.asarray(B, dtype=np.float32)
    Cin = np.ascontiguousarray(np.asarray(Cin, dtype=np.float32))
    assert A.shape == (N_TOTAL, 8) and B.shape == (N_TOTAL, 8)
    assert Cin.shape == (N_TOTAL, 1)

    AB = np.empty((N_TOTAL, 16), dtype=np.float32)
    AB[:, :8] = A
    AB[:, 8:] = B

    nc = _get_nc()

    in_maps = []
    for i in range(N_CORES):
        lo, hi = i * NS, (i + 1) * NS
        in_maps.append({"AB": AB[lo:hi], "Cin": Cin[lo:hi]})

    res = run_bass_kernel_spmd(
        nc, in_maps, core_ids=list(range(N_CORES)), trace=_trace
    )

    sums = np.empty((N_TOTAL, 8), dtype=np.float32)
    carry = np.empty((N_TOTAL, 1), dtype=np.float32)
    for i in range(N_CORES):
        lo, hi = i * NS, (i + 1) * NS
        sums[lo:hi] = res.results[i]["sums"]
        carry[lo:hi] = res.results[i]["carry"]

    if _trace:
        kernel.last_exec_time_ns = res.exec_time_ns
    return sums, carry


kernel.last_exec_time_ns = None


# revision 13
# speedup vs baseline: 1.0627x; 1.0627x over previous
"""8-bit ripple-carry adder on {0,1} floats — Trainium2 Bass kernel.

Problem: A, B [N=2^23, 8] f32 bits (MSB first), Cin [N,1] f32.
reference ripples from bit 7 (LSB) to bit 0 (MSB):
    t = a + b + c ; s = t mod 2 ; c' = t >= 2
Returns (sums [N,8], carry [N,1]) like the reference.

Sharding: batch dim N split evenly across 8 NeuronCores, no communication.

Host side packs A|B row-wise into one [NS,16] tensor per core so each chunk
is a single DMA (the TT ISA slot supports only one sync wait; two separate
loads would land on two DMAHW lanes and need two waits on the consumer).

Per-core layout: rows are processed in chunks of 128*R rows. A chunk of AB
loads contiguously into an SBUF tile [128, 16R] (partition p holds R full
rows). Bit i of each row: A_i = tile[:, i::16], B_i = tile[:, 8+i::16].

Per bit (DVE = vector, ACT = scalar engine):
    ts = A_i + B_i            (DVE tensor_tensor, strided reads)
    t2 = ts + carry           (DVE tensor_tensor, compact)
    carry = t2 >= 2           (DVE tensor_scalar)
    s_i = |sin(pi/2 * t2)|    (ACT sin then abs; exact-enough on {0,1,2,3})
"""

import math
import os

import numpy as np

N_TOTAL = 8388608
N_CORES = 8
NS = N_TOTAL // N_CORES  # rows per core

F32_R = 512  # rows per partition per chunk (f32 path)

_CACHE = {}


def _build_f32(R: int):
    """Wait-slot-safe pipeline (HW compute instructions fit ONE sync wait).

    Tricks:
      - A|B packed host-side into one [NS,16] tensor -> one load DMA/chunk.
      - bf16 intermediates unlock DVE 2x/4x perf modes (values in {0..3}
        are exact in bf16); I/O stays f32.
      - "primer" ops (tiny memset / 16-col ACT copy) absorb WAR-vs-store
        and WAR-vs-other-engine waits so every real op carries only its
        RAW wait. Engine-level sem observation then covers later ops.
    """
    import concourse.tile as tile
    from concourse import bacc, mybir

    f32 = mybir.dt.float32
    bf16 = mybir.dt.bfloat16
    chunk_rows = 128 * R
    n_chunks = NS // chunk_rows
    assert NS % chunk_rows == 0

    nc = bacc.Bacc(None)
    AB = nc.declare_dram_parameter("AB", [NS, 16], f32, isOutput=False)
    Cin = nc.declare_dram_parameter("Cin", [NS, 1], f32, isOutput=False)
    S = nc.declare_dram_parameter("sums", [NS, 8], f32, isOutput=True)
    CO = nc.declare_dram_parameter("carry", [NS, 1], f32, isOutput=True)

    ABv = AB[:].rearrange("(c p r) m -> c p (r m)", p=128, r=R)
    Cv = Cin[:].rearrange("(c p r) m -> c p (r m)", p=128, r=R)
    Sv = S[:].rearrange("(c p r) m -> c p (r m)", p=128, r=R)
    COv = CO[:].rearrange("(c p r) m -> c p (r m)", p=128, r=R)

    HALF_PI = math.pi / 2.0
    Sin = mybir.ActivationFunctionType.Sin
    Abs = mybir.ActivationFunctionType.Abs
    is_ge = mybir.AluOpType.is_ge

    with tile.TileContext(nc) as tc:
        with (
            tc.tile_pool(name="const", bufs=1) as const_pool,
            tc.tile_pool(name="io", bufs=3) as io_pool,
            tc.tile_pool(name="tmp", bufs=3) as tmp_pool,
            tc.tile_pool(name="tsp", bufs=9) as ts_pool,
        ):
            z16 = const_pool.tile([128, 16], f32, tag="z16")
            nc.vector.memset(z16[:], 0.0)
            # Sin bias: shift inputs {0..3} to [-pi, pi/2] where the ACT
            # spline is accurate (sin(3*pi/2) evaluates to 0.9248 raw).
            npi = const_pool.tile([128, 1], f32, tag="npi")
            nc.vector.memset(npi[:], -math.pi)

            for c in range(n_chunks):
                tAB = io_pool.tile([128, 16 * R], f32, tag="AB")
                nc.sync.dma_start(out=tAB[:], in_=ABv[c])
                tC = io_pool.tile([128, R], f32, tag="Cin")
                nc.sync.dma_start(out=tC[:], in_=Cv[c])
                tOUT = io_pool.tile([128, 8 * R], f32, tag="OUT")

                # ACT-side primer: absorbs tOUT's WAR-vs-sums-store wait and
                # WAW-overlaps every abs write region (cols 0..15 hit every
                # i::8 slice).
                nc.scalar.copy(tOUT[:][:, 0:16], z16[:])

                # t2 segments live in one tile; one strided memset absorbs
                # the WAR-vs-sin (ACT) wait for all 8 segments.
                t2a = tmp_pool.tile([128, 8 * R], bf16, tag="t2")
                nc.vector.memset(t2a[:][:, 0 :: R], 0.0)

                # final-carry slot: primer absorbs WAR-vs-carry-store.
                cnf = tmp_pool.tile([128, R], f32, tag="cnf")
                nc.vector.memset(cnf[:][:, 0:1], 0.0)

                # Real data dep on the Cin DMA (carries its DMA wait).
                carry = tmp_pool.tile([128, R], bf16, tag="c0")
                nc.vector.tensor_copy(carry[:], tC[:])

                for k, i in enumerate([7, 6, 5, 4, 3, 2, 1, 0]):
                    ts = ts_pool.tile([128, R], bf16, tag="ts")
                    nc.vector.tensor_add(
                        ts[:], tAB[:][:, i::16], tAB[:][:, 8 + i :: 16]
                    )
                    t2 = t2a[:][:, k * R : (k + 1) * R]
                    nc.vector.tensor_add(t2, ts[:], carry[:])
                    if i > 0:
                        cn = tmp_pool.tile([128, R], bf16, tag="cn")
                    else:
                        cn = cnf
                    nc.vector.tensor_scalar(cn[:], t2, 2.0, None, is_ge)
                    sr = tmp_pool.tile([128, R], bf16, tag="sr")
                    nc.scalar.activation(
                        sr[:], t2, Sin, scale=HALF_PI, bias=npi[:]
                    )
                    nc.scalar.activation(tOUT[:][:, i::8], sr[:], Abs)
                    carry = cn

                # Stores go on OTHER queues than the loads: HWDGE DMAs issue
                # FIFO per engine queue, so a store stalled on compute would
                # block the next chunk's load issue. sums-store rides the
                # scalar (ACT) queue right behind its producer; carry-store
                # uses the idle gpsimd (SWDGE) queue.
                nc.scalar.dma_start(out=Sv[c], in_=tOUT[:])
                nc.gpsimd.dma_start(out=COv[c], in_=cnf[:])
    nc.finalize()
    return nc


def _get_nc():
    key = ("f32", F32_R)
    if key not in _CACHE:
        _CACHE[key] = _build_f32(F32_R)
    return _CACHE[key]


def kernel(A, B, Cin, _trace=False):
    from concourse.bass_utils import run_bass_kernel_spmd

    A = np.asarray(A, dtype=np.float32)
    B = np.asarray(B, dtype=np.float32)
    Cin = np.ascontiguousarray(np.asarray(Cin, dtype=np.float32))
    assert A.shape == (N_TOTAL, 8) and B.shape == (N_TOTAL, 8)
    assert Cin.shape == (N_TOTAL, 1)

    AB = np.empty((N_TOTAL, 16), dtype=np.float32)
    AB[:, :8] = A
    AB[:, 8:] = B

    nc = _get_nc()

    in_maps = []
    for i in range(N_CORES):
        lo, hi = i * NS, (i + 1) * NS
        in_maps.append({"AB": AB[lo:hi], "Cin": Cin[lo:hi]})

    res = run_bass_kernel_spmd(
        nc, in_maps, core_ids=list(range(N_CORES)), trace=_trace
    )

    sums = np.empty((N_TOTAL, 8), dtype=np.float32)
    carry = np.empty((N_TOTAL, 1), dtype=np.float32)
    for i in range(N_CORES):
        lo, hi = i * NS, (i + 1) * NS
        sums[lo:hi] = res.results[i]["sums"]
        carry[lo:hi] = res.results[i]["carry"]

    if _trace:
        kernel.last_exec_time_ns = res.exec_time_ns
    return sums, carry


kernel.last_exec_time_ns = None


# revision 14
# speedup vs baseline: 1.1029x; 1.0378x over previous
"""8-bit ripple-carry adder on {0,1} floats — Trainium2 Bass kernel.

Problem: A, B [N=2^23, 8] f32 bits (MSB first), Cin [N,1] f32.
reference ripples from bit 7 (LSB) to bit 0 (MSB):
    t = a + b + c ; s = t mod 2 ; c' = t >= 2
Returns (sums [N,8], carry [N,1]) like the reference.

Sharding: batch dim N split evenly across 8 NeuronCores, no communication.

Host packs A|B row-wise into one [NS,16] tensor per core so each chunk is a
single load DMA. Loads are SWDGE (gpsimd queue) f32->bf16 cast DMAs: HBM
traffic stays full f32, SBUF halves, and the load queue never sits behind a
store (HWDGE/SWDGE queues issue FIFO per engine; a store waiting on compute
would stall the next chunk's load if they shared a queue). sums-store rides
the scalar queue (right behind its ACT producer), carry-store the sync queue.

Per-core layout: chunks of 128*R rows; an AB chunk is an SBUF tile
[128, 16R] bf16 (partition p holds R rows). Bit i: A_i = t[:, i::16],
B_i = t[:, 8+i::16].

Per bit (DVE = vector, ACT = scalar engine):
    ts = A_i + B_i            DVE (strided reads)
    t2 = ts + carry           DVE (compact bf16, 2x mode)
    carry = t2 >= 2           DVE tensor_scalar (bf16 4x mode)
    s_i:  bits 7..3           ACT sin(pi/2*t2 - pi) then |.| -> tOUT f32
          bits 2..0           DVE STT  s = (carry * -2) + t2  -> tOUT f32
(the sum extraction is split across both engines to balance their loads;
sin needs the -pi bias because the ACT spline is bad at 3*pi/2)

"primer" ops (tiny memset / ACT copy into disjoint columns of tOUT) absorb
the WAR-vs-store semaphore waits so real ops keep a single wait each (HW
compute instructions have ONE sync-wait slot; extra waits cost EventSemaphore
instructions after bacc legalization).
"""

import math
import os

import numpy as np

N_TOTAL = 8388608
N_CORES = 8
NS = N_TOTAL // N_CORES  # rows per core

R = 1024  # rows per partition per chunk
ACT_BITS = (7, 6, 5, 4, 3)  # sum-extraction on ACT; rest on DVE

_CACHE = {}


def _build(R: int):
    import concourse.tile as tile
    from concourse import bacc, mybir

    f32 = mybir.dt.float32
    bf16 = mybir.dt.bfloat16
    chunk_rows = 128 * R
    n_chunks = NS // chunk_rows
    assert NS % chunk_rows == 0

    nc = bacc.Bacc(None)
    AB = nc.declare_dram_parameter("AB", [NS, 16], f32, isOutput=False)
    Cin = nc.declare_dram_parameter("Cin", [NS, 1], f32, isOutput=False)
    S = nc.declare_dram_parameter("sums", [NS, 8], f32, isOutput=True)
    CO = nc.declare_dram_parameter("carry", [NS, 1], f32, isOutput=True)

    ABv = AB[:].rearrange("(c p r) m -> c p (r m)", p=128, r=R)
    Cv = Cin[:].rearrange("(c p r) m -> c p (r m)", p=128, r=R)
    Sv = S[:].rearrange("(c p r) m -> c p (r m)", p=128, r=R)
    COv = CO[:].rearrange("(c p r) m -> c p (r m)", p=128, r=R)

    HALF_PI = math.pi / 2.0
    Sin = mybir.ActivationFunctionType.Sin
    Abs = mybir.ActivationFunctionType.Abs
    is_ge = mybir.AluOpType.is_ge
    mult = mybir.AluOpType.mult
    add = mybir.AluOpType.add

    with tile.TileContext(nc) as tc:
        with (
            tc.tile_pool(name="const", bufs=1) as const_pool,
            tc.tile_pool(name="io", bufs=2) as io_pool,
            tc.tile_pool(name="tmp", bufs=2) as tmp_pool,
            tc.tile_pool(name="tsp", bufs=3) as ts_pool,
        ):
            z16 = const_pool.tile([128, 16], f32, tag="z16")
            nc.vector.memset(z16[:], 0.0)
            # Sin bias: shift {0..3} into [-pi, pi/2] where the spline is
            # accurate (raw sin(3*pi/2) evaluates to 0.9248).
            npi = const_pool.tile([128, 1], f32, tag="npi")
            nc.vector.memset(npi[:], -math.pi)

            for c in range(n_chunks):
                tAB = io_pool.tile([128, 16 * R], bf16, tag="AB")
                nc.gpsimd.dma_start(out=tAB[:], in_=ABv[c])
                tC = io_pool.tile([128, R], bf16, tag="Cin")
                nc.gpsimd.dma_start(out=tC[:], in_=Cv[c])
                tOUT = io_pool.tile([128, 8 * R], f32, tag="OUT")

                # Disjoint-column store-WAR absorbers for tOUT: ACT writes
                # cols {3..7} mod 8, DVE writes cols {0..2} mod 8.
                nc.scalar.copy(tOUT[:][:, 3:8], z16[:][:, 0:5])
                nc.vector.memset(tOUT[:][:, 0:3], 0.0)

                # t2 segments in one tile; strided memset absorbs the
                # WAR-vs-ACT-sin wait for all segments at once.
                t2a = tmp_pool.tile([128, 8 * R], bf16, tag="t2")
                nc.vector.memset(t2a[:][:, 0::R], 0.0)

                # final-carry slot: primer absorbs WAR-vs-carry-store.
                cnf = tmp_pool.tile([128, R], f32, tag="cnf")
                nc.vector.memset(cnf[:][:, 0:1], 0.0)

                # Real data dep on the Cin DMA (isolates its DMA wait).
                carry = tmp_pool.tile([128, R], bf16, tag="c0")
                nc.vector.tensor_copy(carry[:], tC[:])

                for k, i in enumerate([7, 6, 5, 4, 3, 2, 1, 0]):
                    ts = ts_pool.tile([128, R], bf16, tag="ts")
                    nc.vector.tensor_add(
                        ts[:], tAB[:][:, i::16], tAB[:][:, 8 + i :: 16]
                    )
                    t2 = t2a[:][:, k * R : (k + 1) * R]
                    nc.vector.tensor_add(t2, ts[:], carry[:])
                    if i > 0:
                        cn = tmp_pool.tile([128, R], bf16, tag="cn")
                    else:
                        cn = cnf
                    nc.vector.tensor_scalar(cn[:], t2, 2.0, None, is_ge)

                    if i in ACT_BITS:
                        sr = tmp_pool.tile([128, R], bf16, tag="sr")
                        nc.scalar.activation(
                            sr[:], t2, Sin, scale=HALF_PI, bias=npi[:]
                        )
                        nc.scalar.activation(tOUT[:][:, i::8], sr[:], Abs)
                    else:
                        nc.vector.scalar_tensor_tensor(
                            tOUT[:][:, i::8], cn[:], -2.0, t2, mult, add
                        )
                    carry = cn

                nc.scalar.dma_start(out=Sv[c], in_=tOUT[:])
                nc.sync.dma_start(out=COv[c], in_=cnf[:])
    nc.finalize()
    return nc


def _get_nc():
    key = ("v3", R)
    if key not in _CACHE:
        _CACHE[key] = _build(R)
    return _CACHE[key]


def kernel(A, B, Cin, _trace=False):
    from concourse.bass_utils import run_bass_kernel_spmd

    A = np.asarray(A, dtype=np.float32)
    B = np.asarray(B, dtype=np.float32)
    Cin = np.ascontiguousarray(np.asarray(Cin, dtype=np.float32))
    assert A.shape == (N_TOTAL, 8) and B.shape == (N_TOTAL, 8)
    assert Cin.shape == (N_TOTAL, 1)

    AB = np.empty((N_TOTAL, 16), dtype=np.float32)
    AB[:, :8] = A
    AB[:, 8:] = B

    nc = _get_nc()

    in_maps = []
    for i in range(N_CORES):
        lo, hi = i * NS, (i + 1) * NS
        in_maps.append({"AB": AB[lo:hi], "Cin": Cin[lo:hi]})

    res = run_bass_kernel_spmd(
        nc, in_maps, core_ids=list(range(N_CORES)), trace=_trace
    )

    sums = np.empty((N_TOTAL, 8), dtype=np.float32)
    carry = np.empty((N_TOTAL, 1), dtype=np.float32)
    for i in range(N_CORES):
        lo, hi = i * NS, (i + 1) * NS
        sums[lo:hi] = res.results[i]["sums"]
        carry[lo:hi] = res.results[i]["carry"]

    if _trace:
        kernel.last_exec_time_ns = res.exec_time_ns
    return sums, carry


kernel.last_exec_time_ns = None


# revision 15
# speedup vs baseline: 1.2676x; 1.1493x over previous
"""8-bit ripple-carry adder on {0,1} floats — Trainium2 Bass kernel.

Problem: A, B [N=2^23, 8] f32 bits (MSB first), Cin [N,1] f32.
reference ripples from bit 7 (LSB) to bit 0 (MSB):
    t = a + b + c ; s = t mod 2 ; c' = t >= 2
Returns (sums [N,8], carry [N,1]) like the reference.

Sharding: batch dim N split evenly across 8 NeuronCores, no communication.

Key measured facts driving the design (trn2, DVE @0.96GHz):
  - compact bf16 tensor_tensor = 0.52 cyc/elem (2x mode); f32 / strided = 1
  - single-element strided WRITES cost ~2 cyc/elem (reads are free)
  - the ripple chain is serial: each dependent DVE op adds a ~0.3-0.4us
    semaphore stall, so two independent chains are interleaved (chunk pairs)
    to fill the gaps
  - HWDGE/SWDGE queues issue FIFO per engine: a store waiting on compute
    must never sit in front of a load. Queue map: gpsimd = AB cast-loads
    (f32->bf16; HBM side still reads full f32), sync = Cin loads,
    scalar = ACT ops + both stores, vector = compute.

Host packs A|B row-wise into one [NS,16] tensor per core so each chunk is a
single load DMA. Chunks are 128*R rows as SBUF tiles [128,16R] bf16
(partition p holds R rows; bit i of A = t[:, i::16], of B = t[:, 8+i::16]).

Per chunk: U = A + B for all bits in ONE 2x-mode op, then per bit
    t2 = U_i + carry          DVE (strided read, compact bf16 write)
    carry = t2 >= 2           DVE tensor_scalar (bf16 4x mode)
    s_i:  bits 7..3           ACT sin(pi/2*t2 - pi) then |.| (the -pi bias
                              avoids the bad spline region at 3*pi/2)
          bits 2..0           DVE STT  s = (carry * -2) + t2
Tiny "primer" ops (memset / ACT copy into disjoint columns) absorb
WAR-vs-store waits so real ops keep one semaphore wait each (the HW wait
slot; extras become EventSemaphore instructions via bacc).
"""

import math
import os

import numpy as np

N_TOTAL = 8388608
N_CORES = 8
NS = N_TOTAL // N_CORES  # rows per core

R = 512  # rows per partition per chunk
ACT_BITS = (7, 6, 5, 4, 3)  # sum-extraction on ACT; rest on DVE

_CACHE = {}


def _build(R: int):
    import concourse.tile as tile
    from concourse import bacc, mybir

    f32 = mybir.dt.float32
    bf16 = mybir.dt.bfloat16
    chunk_rows = 128 * R
    n_chunks = NS // chunk_rows
    assert NS % chunk_rows == 0 and n_chunks % 2 == 0

    nc = bacc.Bacc(None)
    AB = nc.declare_dram_parameter("AB", [NS, 16], f32, isOutput=False)
    Cin = nc.declare_dram_parameter("Cin", [NS, 1], f32, isOutput=False)
    S = nc.declare_dram_parameter("sums", [NS, 8], f32, isOutput=True)
    CO = nc.declare_dram_parameter("carry", [NS, 1], f32, isOutput=True)

    ABv = AB[:].rearrange("(c p r) m -> c p (r m)", p=128, r=R)
    Cv = Cin[:].rearrange("(c p r) m -> c p (r m)", p=128, r=R)
    Sv = S[:].rearrange("(c p r) m -> c p (r m)", p=128, r=R)
    COv = CO[:].rearrange("(c p r) m -> c p (r m)", p=128, r=R)

    HALF_PI = math.pi / 2.0
    Sin = mybir.ActivationFunctionType.Sin
    Abs = mybir.ActivationFunctionType.Abs
    is_ge = mybir.AluOpType.is_ge
    mult = mybir.AluOpType.mult
    add = mybir.AluOpType.add

    with tile.TileContext(nc) as tc:
        with (
            tc.tile_pool(name="const", bufs=1) as const_pool,
            tc.tile_pool(name="io", bufs=3) as io_pool,
            tc.tile_pool(name="tmp", bufs=3) as tmp_pool,
            tc.tile_pool(name="cnp", bufs=4) as cn_pool,
        ):
            z16 = const_pool.tile([128, 16], f32, tag="z16")
            nc.vector.memset(z16[:], 0.0)
            npi = const_pool.tile([128, 1], f32, tag="npi")
            nc.vector.memset(npi[:], -math.pi)

            class Chunk:
                pass

            def start_chunk(c):
                ch = Chunk()
                ch.c = c
                ch.tAB = io_pool.tile([128, 16 * R], bf16, tag="AB")
                nc.gpsimd.dma_start(out=ch.tAB[:], in_=ABv[c])
                ch.tC = io_pool.tile([128, R], f32, tag="Cin")
                nc.sync.dma_start(out=ch.tC[:], in_=Cv[c])
                ch.tOUT = io_pool.tile([128, 8 * R], f32, tag="OUT")
                # disjoint store-WAR absorbers (ACT cols 3..7, DVE cols 0..2)
                nc.scalar.copy(ch.tOUT[:][:, 3:8], z16[:][:, 0:5])
                nc.vector.memset(ch.tOUT[:][:, 0:3], 0.0)
                ch.t2a = tmp_pool.tile([128, 8 * R], bf16, tag="t2")
                nc.vector.memset(ch.t2a[:][:, 0::R], 0.0)
                ch.cnf = tmp_pool.tile([128, R], f32, tag="cnf")
                nc.vector.memset(ch.cnf[:][:, 0:1], 0.0)
                ch.carry = tmp_pool.tile([128, R], bf16, tag="c0")
                nc.vector.tensor_copy(ch.carry[:], ch.tC[:])
                # U = A + B for all 8 bit positions in one 2x-mode op
                ch.U = tmp_pool.tile([128, 8 * R], bf16, tag="U")
                abv = ch.tAB[:].rearrange("p (r m) -> p r m", m=16)
                uv = ch.U[:].rearrange("p (r m) -> p r m", m=8)
                nc.vector.tensor_tensor(
                    uv[:, :, 0:8], abv[:, :, 0:8], abv[:, :, 8:16], add
                )
                return ch

            def chain_step(ch, k, i):
                t2 = ch.t2a[:][:, k * R : (k + 1) * R]
                nc.vector.tensor_add(t2, ch.U[:][:, i::8], ch.carry[:])
                if i > 0:
                    cn = cn_pool.tile([128, R], bf16, tag="cn")
                else:
                    cn = ch.cnf
                nc.vector.tensor_scalar(cn[:], t2, 2.0, None, is_ge)
                if i in ACT_BITS:
                    sr = cn_pool.tile([128, R], bf16, tag="sr")
                    nc.scalar.activation(
                        sr[:], t2, Sin, scale=HALF_PI, bias=npi[:]
                    )
                    nc.scalar.activation(ch.tOUT[:][:, i::8], sr[:], Abs)
                else:
                    nc.vector.scalar_tensor_tensor(
                        ch.tOUT[:][:, i::8], cn[:], -2.0, t2, mult, add
                    )
                ch.carry = cn

            def finish_chunk(ch):
                nc.scalar.dma_start(out=Sv[ch.c], in_=ch.tOUT[:])
                nc.scalar.dma_start(out=COv[ch.c], in_=ch.cnf[:])

            # process chunks in pairs; the two ripple chains interleave on
            # DVE so one chain's semaphore stalls hide behind the other's
            # compute.
            for p in range(n_chunks // 2):
                cha = start_chunk(2 * p)
                chb = start_chunk(2 * p + 1)
                for k, i in enumerate([7, 6, 5, 4, 3, 2, 1, 0]):
                    chain_step(cha, k, i)
                    chain_step(chb, k, i)
                finish_chunk(cha)
                finish_chunk(chb)
    nc.finalize()
    return nc


def _get_nc():
    key = ("v4", R)
    if key not in _CACHE:
        _CACHE[key] = _build(R)
    return _CACHE[key]


def kernel(A, B, Cin, _trace=False):
    from concourse.bass_utils import run_bass_kernel_spmd

    A = np.asarray(A, dtype=np.float32)
    B = np.asarray(B, dtype=np.float32)
    Cin = np.ascontiguousarray(np.asarray(Cin, dtype=np.float32))
    assert A.shape == (N_TOTAL, 8) and B.shape == (N_TOTAL, 8)
    assert Cin.shape == (N_TOTAL, 1)

    AB = np.empty((N_TOTAL, 16), dtype=np.float32)
    AB[:, :8] = A
    AB[:, 8:] = B

    nc = _get_nc()

    in_maps = []
    for i in range(N_CORES):
        lo, hi = i * NS, (i + 1) * NS
        in_maps.append({"AB": AB[lo:hi], "Cin": Cin[lo:hi]})

    res = run_bass_kernel_spmd(
        nc, in_maps, core_ids=list(range(N_CORES)), trace=_trace
    )

    sums = np.empty((N_TOTAL, 8), dtype=np.float32)
    carry = np.empty((N_TOTAL, 1), dtype=np.float32)
    for i in range(N_CORES):
        lo, hi = i * NS, (i + 1) * NS
        sums[lo:hi] = res.results[i]["sums"]
        carry[lo:hi] = res.results[i]["carry"]

    if _trace:
        kernel.last_exec_time_ns = res.exec_time_ns
    return sums, carry


kernel.last_exec_time_ns = None


# revision 16
# speedup vs baseline: 1.2964x; 1.0227x over previous
"""8-bit ripple-carry adder on {0,1} floats — Trainium2 Bass kernel.

Problem: A, B [N=2^23, 8] f32 bits (MSB first), Cin [N,1] f32.
reference ripples from bit 7 (LSB) to bit 0 (MSB):
    t = a + b + c ; s = t mod 2 ; c' = t >= 2
Returns (sums [N,8], carry [N,1]) like the reference.

Sharding: batch dim N split evenly across 8 NeuronCores, no communication.

Key measured facts driving the design (trn2, DVE @0.96GHz):
  - compact bf16 tensor_tensor = 0.52 cyc/elem (2x mode); f32 / strided = 1
  - single-element strided WRITES cost ~2 cyc/elem (reads are free)
  - the ripple chain is serial: each dependent DVE op adds a ~0.3-0.4us
    semaphore stall, so two independent chains are interleaved (chunk pairs)
    to fill the gaps
  - HWDGE/SWDGE queues issue FIFO per engine: a store waiting on compute
    must never sit in front of a load. Queue map: gpsimd = AB cast-loads
    (f32->bf16; HBM side still reads full f32), sync = Cin loads,
    scalar = ACT ops + both stores, vector = compute.

Host packs A|B row-wise into one [NS,16] tensor per core so each chunk is a
single load DMA. Chunks are 128*R rows as SBUF tiles [128,16R] bf16
(partition p holds R rows; bit i of A = t[:, i::16], of B = t[:, 8+i::16]).

Per chunk: U = A + B for all bits in ONE 2x-mode op, then per bit
    t2 = U_i + carry          DVE (strided read, compact bf16 write)
    carry = t2 >= 2           DVE tensor_scalar (bf16 4x mode)
    s_i:  bits 7..3           ACT sin(pi/2*t2 - pi) then |.| (the -pi bias
                              avoids the bad spline region at 3*pi/2)
          bits 2..0           DVE STT  s = (carry * -2) + t2
Tiny "primer" ops (memset / ACT copy into disjoint columns) absorb
WAR-vs-store waits so real ops keep one semaphore wait each (the HW wait
slot; extras become EventSemaphore instructions via bacc).
"""

import math
import os

import numpy as np

N_TOTAL = 8388608
N_CORES = 8
NS = N_TOTAL // N_CORES  # rows per core

R = 512  # rows per partition per chunk
ACT_BITS = (7, 6, 5, 4, 3)  # sum-extraction on ACT; rest on DVE

_CACHE = {}


def _build(R: int):
    import concourse.tile as tile
    from concourse import bacc, mybir

    f32 = mybir.dt.float32
    bf16 = mybir.dt.bfloat16
    chunk_rows = 128 * R
    n_chunks = NS // chunk_rows
    assert NS % chunk_rows == 0 and n_chunks % 2 == 0

    nc = bacc.Bacc(None)
    AB = nc.declare_dram_parameter("AB", [NS, 16], f32, isOutput=False)
    Cin = nc.declare_dram_parameter("Cin", [NS, 1], f32, isOutput=False)
    S = nc.declare_dram_parameter("sums", [NS, 8], f32, isOutput=True)
    CO = nc.declare_dram_parameter("carry", [NS, 1], f32, isOutput=True)

    ABv = AB[:].rearrange("(c p r) m -> c p (r m)", p=128, r=R)
    Cv = Cin[:].rearrange("(c p r) m -> c p (r m)", p=128, r=R)
    Sv = S[:].rearrange("(c p r) m -> c p (r m)", p=128, r=R)
    COv = CO[:].rearrange("(c p r) m -> c p (r m)", p=128, r=R)

    HALF_PI = math.pi / 2.0
    Sin = mybir.ActivationFunctionType.Sin
    Abs = mybir.ActivationFunctionType.Abs
    is_ge = mybir.AluOpType.is_ge
    mult = mybir.AluOpType.mult
    add = mybir.AluOpType.add

    with tile.TileContext(nc) as tc:
        with (
            tc.tile_pool(name="const", bufs=1) as const_pool,
            tc.tile_pool(name="ab", bufs=2) as ab_pool,
            tc.tile_pool(name="io", bufs=3) as io_pool,
            tc.tile_pool(name="tmp", bufs=3) as tmp_pool,
            tc.tile_pool(name="cnp", bufs=4) as cn_pool,
        ):
            z16 = const_pool.tile([128, 16], f32, tag="z16")
            nc.vector.memset(z16[:], 0.0)
            npi = const_pool.tile([128, 1], f32, tag="npi")
            nc.vector.memset(npi[:], -math.pi)

            class Chunk:
                pass

            def start_chunk(c):
                ch = Chunk()
                ch.c = c
                ch.tAB = ab_pool.tile([128, 16 * R], f32, tag="AB")
                nc.sync.dma_start(out=ch.tAB[:], in_=ABv[c])
                ch.tC = io_pool.tile([128, R], f32, tag="Cin")
                nc.sync.dma_start(out=ch.tC[:], in_=Cv[c])
                ch.tOUT = io_pool.tile([128, 8 * R], f32, tag="OUT")
                # disjoint store-WAR absorbers (ACT cols 3..7, DVE cols 0..2)
                nc.scalar.copy(ch.tOUT[:][:, 3:8], z16[:][:, 0:5])
                nc.vector.memset(ch.tOUT[:][:, 0:3], 0.0)
                ch.t2a = tmp_pool.tile([128, 8 * R], bf16, tag="t2")
                nc.vector.memset(ch.t2a[:][:, 0::R], 0.0)
                ch.cnf = tmp_pool.tile([128, R], f32, tag="cnf")
                nc.vector.memset(ch.cnf[:][:, 0:1], 0.0)
                ch.carry = tmp_pool.tile([128, R], bf16, tag="c0")
                nc.vector.tensor_copy(ch.carry[:], ch.tC[:])
                # U = A + B for all 8 bit positions in one 2x-mode op
                ch.U = tmp_pool.tile([128, 8 * R], bf16, tag="U")
                abv = ch.tAB[:].rearrange("p (r m) -> p r m", m=16)
                uv = ch.U[:].rearrange("p (r m) -> p r m", m=8)
                nc.vector.tensor_tensor(
                    uv[:, :, 0:8], abv[:, :, 0:8], abv[:, :, 8:16], add
                )
                return ch

            def chain_step(ch, k, i):
                t2 = ch.t2a[:][:, k * R : (k + 1) * R]
                nc.vector.tensor_add(t2, ch.U[:][:, i::8], ch.carry[:])
                if i > 0:
                    cn = cn_pool.tile([128, R], bf16, tag="cn")
                else:
                    cn = ch.cnf
                nc.vector.tensor_scalar(cn[:], t2, 2.0, None, is_ge)
                if i in ACT_BITS:
                    sr = cn_pool.tile([128, R], bf16, tag="sr")
                    nc.scalar.activation(
                        sr[:], t2, Sin, scale=HALF_PI, bias=npi[:]
                    )
                    nc.scalar.activation(ch.tOUT[:][:, i::8], sr[:], Abs)
                else:
                    nc.vector.scalar_tensor_tensor(
                        ch.tOUT[:][:, i::8], cn[:], -2.0, t2, mult, add
                    )
                ch.carry = cn

            def finish_chunk(ch):
                nc.scalar.dma_start(out=Sv[ch.c], in_=ch.tOUT[:])
                nc.scalar.dma_start(out=COv[ch.c], in_=ch.cnf[:])

            # process chunks in pairs; the two ripple chains interleave on
            # DVE so one chain's semaphore stalls hide behind the other's
            # compute.
            for p in range(n_chunks // 2):
                cha = start_chunk(2 * p)
                chb = start_chunk(2 * p + 1)
                for k, i in enumerate([7, 6, 5, 4, 3, 2, 1, 0]):
                    chain_step(cha, k, i)
                    chain_step(chb, k, i)
                finish_chunk(cha)
                finish_chunk(chb)
    nc.finalize()
    return nc


def _get_nc():
    key = ("v5", R)
    if key not in _CACHE:
        _CACHE[key] = _build(R)
    return _CACHE[key]


def kernel(A, B, Cin, _trace=False):
    from concourse.bass_utils import run_bass_kernel_spmd

    A = np.asarray(A, dtype=np.float32)
    B = np.asarray(B, dtype=np.float32)
    Cin = np.ascontiguousarray(np.asarray(Cin, dtype=np.float32))
    assert A.shape == (N_TOTAL, 8) and B.shape == (N_TOTAL, 8)
    assert Cin.shape == (N_TOTAL, 1)

    AB = np.empty((N_TOTAL, 16), dtype=np.float32)
    AB[:, :8] = A
    AB[:, 8:] = B

    nc = _get_nc()

    in_maps = []
    for i in range(N_CORES):
        lo, hi = i * NS, (i + 1) * NS
        in_maps.append({"AB": AB[lo:hi], "Cin": Cin[lo:hi]})

    res = run_bass_kernel_spmd(
        nc, in_maps, core_ids=list(range(N_CORES)), trace=_trace
    )

    sums = np.empty((N_TOTAL, 8), dtype=np.float32)
    carry = np.empty((N_TOTAL, 1), dtype=np.float32)
    for i in range(N_CORES):
        lo, hi = i * NS, (i + 1) * NS
        sums[lo:hi] = res.results[i]["sums"]
        carry[lo:hi] = res.results[i]["carry"]

    if _trace:
        kernel.last_exec_time_ns = res.exec_time_ns
    return sums, carry


kernel.last_exec_time_ns = None


# revision 18
# speedup vs baseline: 1.3079x; 1.0089x over previous
"""8-bit ripple-carry adder on {0,1} floats — Trainium2 Bass kernel.

Problem: A, B [N=2^23, 8] f32 bits (MSB first), Cin [N,1] f32.
reference ripples from bit 7 (LSB) to bit 0 (MSB):
    t = a + b + c ; s = t mod 2 ; c' = t >= 2
Returns (sums [N,8], carry [N,1]) like the reference.

Sharding: batch dim N split evenly across 8 NeuronCores, no communication.

Key measured facts driving the design (trn2, DVE @0.96GHz):
  - compact bf16 tensor_tensor = 0.52 cyc/elem (2x mode); f32 / strided = 1
  - single-element strided WRITES cost ~2 cyc/elem (reads are free)
  - the ripple chain is serial: each dependent DVE op adds a ~0.3-0.4us
    semaphore stall, so two independent chains are interleaved (chunk pairs)
    to fill the gaps
  - HWDGE/SWDGE queues issue FIFO per engine: a store waiting on compute
    must never sit in front of a load. Queue map: gpsimd = AB cast-loads
    (f32->bf16; HBM side still reads full f32), sync = Cin loads,
    scalar = ACT ops + both stores, vector = compute.

Host packs A|B row-wise into one [NS,16] tensor per core so each chunk is a
single load DMA. Chunks are 128*R rows as SBUF tiles [128,16R] bf16
(partition p holds R rows; bit i of A = t[:, i::16], of B = t[:, 8+i::16]).

Per chunk: U = A + B for all bits in ONE 2x-mode op, then per bit
    t2 = U_i + carry          DVE (strided read, compact bf16 write)
    carry = t2 >= 2           DVE tensor_scalar (bf16 4x mode)
    s_i:  bits 7..3           ACT sin(pi/2*t2 - pi) then |.| (the -pi bias
                              avoids the bad spline region at 3*pi/2)
          bits 2..0           DVE STT  s = (carry * -2) + t2
Tiny "primer" ops (memset / ACT copy into disjoint columns) absorb
WAR-vs-store waits so real ops keep one semaphore wait each (the HW wait
slot; extras become EventSemaphore instructions via bacc).
"""

import math
import os

import numpy as np

N_TOTAL = 8388608
N_CORES = 8
NS = N_TOTAL // N_CORES  # rows per core

R = 512  # rows per partition per chunk
ACT_BITS = (7, 6, 5, 4, 3)  # sum-extraction on ACT; rest on DVE

_CACHE = {}


def _build(R: int):
    import concourse.tile as tile
    from concourse import bacc, mybir

    f32 = mybir.dt.float32
    bf16 = mybir.dt.bfloat16
    chunk_rows = 128 * R
    n_chunks = NS // chunk_rows
    assert NS % chunk_rows == 0 and n_chunks % 2 == 0

    nc = bacc.Bacc(None)
    AB = nc.declare_dram_parameter("AB", [NS, 16], f32, isOutput=False)
    Cin = nc.declare_dram_parameter("Cin", [NS, 1], f32, isOutput=False)
    S = nc.declare_dram_parameter("sums", [NS, 8], f32, isOutput=True)
    CO = nc.declare_dram_parameter("carry", [NS, 1], f32, isOutput=True)

    ABv = AB[:].rearrange("(c p r) m -> c p (r m)", p=128, r=R)
    Cv = Cin[:].rearrange("(c p r) m -> c p (r m)", p=128, r=R)
    Sv = S[:].rearrange("(c p r) m -> c p (r m)", p=128, r=R)
    COv = CO[:].rearrange("(c p r) m -> c p (r m)", p=128, r=R)

    HALF_PI = math.pi / 2.0
    Sin = mybir.ActivationFunctionType.Sin
    Abs = mybir.ActivationFunctionType.Abs
    is_ge = mybir.AluOpType.is_ge
    mult = mybir.AluOpType.mult
    add = mybir.AluOpType.add

    with tile.TileContext(nc) as tc:
        with (
            tc.tile_pool(name="const", bufs=1) as const_pool,
            tc.tile_pool(name="ab", bufs=2) as ab_pool,
            tc.tile_pool(name="io", bufs=3) as io_pool,
            tc.tile_pool(name="tmp", bufs=3) as tmp_pool,
            tc.tile_pool(name="cnp", bufs=6) as cn_pool,
        ):
            z16 = const_pool.tile([128, 16], f32, tag="z16")
            nc.vector.memset(z16[:], 0.0)
            npi = const_pool.tile([128, 1], f32, tag="npi")
            nc.vector.memset(npi[:], -math.pi)

            class Chunk:
                pass

            def start_chunk(c):
                ch = Chunk()
                ch.c = c
                ch.tAB = ab_pool.tile([128, 16 * R], f32, tag="AB")
                nc.sync.dma_start(out=ch.tAB[:], in_=ABv[c])
                ch.tC = io_pool.tile([128, R], f32, tag="Cin")
                nc.sync.dma_start(out=ch.tC[:], in_=Cv[c])
                ch.tOUT = io_pool.tile([128, 8 * R], f32, tag="OUT")
                # disjoint store-WAR absorbers (ACT cols 3..7, DVE cols 0..2)
                nc.scalar.copy(ch.tOUT[:][:, 3:8], z16[:][:, 0:5])
                nc.vector.memset(ch.tOUT[:][:, 0:3], 0.0)
                ch.t2a = tmp_pool.tile([128, 8 * R], bf16, tag="t2")
                nc.vector.memset(ch.t2a[:][:, 0::R], 0.0)
                ch.cnf = tmp_pool.tile([128, R], f32, tag="cnf")
                nc.vector.memset(ch.cnf[:][:, 0:1], 0.0)
                ch.carry = tmp_pool.tile([128, R], bf16, tag="c0")
                nc.vector.tensor_copy(ch.carry[:], ch.tC[:])
                # U = A + B for all 8 bit positions in one 2x-mode op
                ch.U = tmp_pool.tile([128, 8 * R], bf16, tag="U")
                abv = ch.tAB[:].rearrange("p (r m) -> p r m", m=16)
                uv = ch.U[:].rearrange("p (r m) -> p r m", m=8)
                nc.vector.tensor_tensor(
                    uv[:, :, 0:8], abv[:, :, 0:8], abv[:, :, 8:16], add
                )
                return ch

            def chain_step(ch, k, i):
                t2 = ch.t2a[:][:, k * R : (k + 1) * R]
                nc.vector.tensor_add(t2, ch.U[:][:, i::8], ch.carry[:])
                if i > 0:
                    cn = cn_pool.tile([128, R], bf16, tag="cn")
                else:
                    cn = ch.cnf
                nc.vector.tensor_scalar(cn[:], t2, 2.0, None, is_ge)
                if i in ACT_BITS:
                    sr = cn_pool.tile([128, R], bf16, tag="sr")
                    nc.scalar.activation(
                        sr[:], t2, Sin, scale=HALF_PI, bias=npi[:]
                    )
                    nc.scalar.activation(ch.tOUT[:][:, i::8], sr[:], Abs)
                else:
                    nc.vector.scalar_tensor_tensor(
                        ch.tOUT[:][:, i::8], cn[:], -2.0, t2, mult, add
                    )
                ch.carry = cn

            def finish_chunk(ch):
                nc.scalar.dma_start(out=Sv[ch.c], in_=ch.tOUT[:])
                nc.scalar.dma_start(out=COv[ch.c], in_=ch.cnf[:])

            # Rolling 2-deep software pipeline with half-chain stagger: one
            # chunk rides bits 7..4 while the other rides 3..0, so chain
            # semaphore stalls, ACT work, loads and stores all spread out
            # instead of bursting at pair boundaries.
            bits = [7, 6, 5, 4, 3, 2, 1, 0]
            active = []
            next_c = 0

            def advance(ch):
                k = ch.step
                chain_step(ch, k, bits[k])
                ch.step += 1
                return ch.step == 8

            ch = start_chunk(next_c)
            ch.step = 0
            active.append(ch)
            next_c += 1
            for _ in range(4):
                advance(active[0])
            ch = start_chunk(next_c)
            ch.step = 0
            active.append(ch)
            next_c += 1

            while active:
                for ch in list(active):
                    if advance(ch):
                        finish_chunk(ch)
                        active.remove(ch)
                        if next_c < n_chunks:
                            nch = start_chunk(next_c)
                            nch.step = 0
                            active.append(nch)
                            next_c += 1
    nc.finalize()
    return nc


def _get_nc():
    key = ("v5", R)
    if key not in _CACHE:
        _CACHE[key] = _build(R)
    return _CACHE[key]


def kernel(A, B, Cin, _trace=False):
    from concourse.bass_utils import run_bass_kernel_spmd

    A = np.asarray(A, dtype=np.float32)
    B = np.asarray(B, dtype=np.float32)
    Cin = np.ascontiguousarray(np.asarray(Cin, dtype=np.float32))
    assert A.shape == (N_TOTAL, 8) and B.shape == (N_TOTAL, 8)
    assert Cin.shape == (N_TOTAL, 1)

    AB = np.empty((N_TOTAL, 16), dtype=np.float32)
    AB[:, :8] = A
    AB[:, 8:] = B

    nc = _get_nc()

    in_maps = []
    for i in range(N_CORES):
        lo, hi = i * NS, (i + 1) * NS
        in_maps.append({"AB": AB[lo:hi], "Cin": Cin[lo:hi]})

    res = run_bass_kernel_spmd(
        nc, in_maps, core_ids=list(range(N_CORES)), trace=_trace
    )

    sums = np.empty((N_TOTAL, 8), dtype=np.float32)
    carry = np.empty((N_TOTAL, 1), dtype=np.float32)
    for i in range(N_CORES):
        lo, hi = i * NS, (i + 1) * NS
        sums[lo:hi] = res.results[i]["sums"]
        carry[lo:hi] = res.results[i]["carry"]

    if _trace:
        kernel.last_exec_time_ns = res.exec_time_ns
    return sums, carry


kernel.last_exec_time_ns = None
